# revision 5
# baseline (speedup 1.0000x reference)
"""Debayer3x3 Trainium2 Bass kernel.

Reference computation (per image, H=W=2048, f32):
  xpad = reflect-pad(x, 1)
  cross = 0.25*(up+down+left+right), diag = 0.25*(4 diagonals),
  hz = 0.5*(left+right), vt = 0.5*(up+down)
  R = [[x, hz], [vt, diag]]  (2x2 row/col parity pattern)
  G = [[cross, x], [x, cross]]
  B = [[diag, vt], [hz, x]]

Strategy: pure data parallel, 1 image per NeuronCore (batch 8 / 8 cores).
Per core, layout tiles with K=16 consecutive image rows per SBUF partition
(128 partitions x 16 rows = 2048 rows) so every vertical neighbor access is a
free-dim shift (compute engines cannot shift across partitions: SBUF APs must
start at partition 0/32/64/96).  Each partition loads K+2 rows (1-row halo on
each side) and Wc+2 cols (1-col halo) so reflect padding is resolved at load
time.  Work planes:
  X  *= 0.25                      (in-place prescale, DVE tensor_scalar 2x)
  Hq[jx,c] = X[jx,c] + X[jx,c+2]  (= 0.25*(left+right) at row jx, col c)
  Vq[j,c]  = X[j,c+1] + X[j+2,c+1](= 0.25*(up+down) at out row j, col c)
Then every output quadrant is either a scaled copy (ACT, scale folds the
0.25 prescale back out) or a single strided tensor_tensor add:
  hz = 2*Hq, vt = 2*Vq, diag = Hq[j]+Hq[j+2], cross = Vq[j]+Hq[j+1], x = 4*X.
"""

from contextlib import ExitStack

import numpy as np

H, W = 2048, 2048
K = 16          # image rows per partition
WC = 256        # column chunk width
N_CORES = 8

_compiled = {}


def _build(nc_mod, tile_mod, mybir, h, w, k, wc, bacc_mod=None):
    """Emit the debayer program for one core into a fresh Bass object."""
    bass = nc_mod
    p = h // k
    assert p <= 128 and h % k == 0 and w % wc == 0 and k % 2 == 0 and wc % 2 == 0
    nchunks = w // wc
    dt = mybir.dt.float32
    Copy = mybir.ActivationFunctionType.Copy

    nc = bass.Bass() if bacc_mod is None else bacc_mod.Bacc("TRN2")
    x = nc.dram_tensor("x", [h, w], dt, kind="ExternalInput")
    out = nc.dram_tensor("out", [3, h, w], dt, kind="ExternalOutput")
    xr = x.rearrange("(p k) w -> p k w", k=k)
    outr = out.rearrange("c (p k) w -> c p k w", k=k)

    with tile_mod.TileContext(nc) as tc:
        with ExitStack() as ctx:
            xpool = ctx.enter_context(tc.tile_pool(name="xin", bufs=2))
            mpool = ctx.enter_context(tc.tile_pool(name="mid", bufs=1))
            opool = ctx.enter_context(tc.tile_pool(name="outp", bufs=2))

            # row slices into out tiles (j) and into X/Hq (jx = j+1 is the
            # same image row; Hq/X rows 0..k+1 carry the halo)
            je, jo = slice(0, k, 2), slice(1, k, 2)
            jxe, jxo = slice(1, k + 1, 2), slice(2, k + 2, 2)
            jm_e, jp_e = slice(0, k, 2), slice(2, k + 2, 2)      # j, j+2 for j even
            jm_o, jp_o = slice(1, k, 2), slice(3, k + 2, 2)      # j, j+2 for j odd

            for ci in range(nchunks):
                c0 = ci * wc
                # X cols 0..wc+1 <-> image cols c0-1 .. c0+wc
                lo = 1 if ci == 0 else 0
                hi = wc + 1 if ci == nchunks - 1 else wc + 2
                dlo = c0 - 1 + lo
                ncol = hi - lo

                X = xpool.tile([p, k + 2, wc + 2], dt)
                # main body rows
                nc.sync.dma_start(X[:, 1 : k + 1, lo:hi], xr[:, :, dlo : dlo + ncol])
                # top halo: partition q row 0 <- image row k*q - 1 (q >= 1)
                nc.sync.dma_start(
                    X[1:p, 0:1, lo:hi], xr[0 : p - 1, k - 1 : k, dlo : dlo + ncol]
                )
                # bottom halo: partition q row k+1 <- image row k*q + k (q <= p-2)
                nc.sync.dma_start(
                    X[0 : p - 1, k + 1 : k + 2, lo:hi], xr[1:p, 0:1, dlo : dlo + ncol]
                )
                # reflect rows: image row -1 := row 1 ; row h := row h-2
                nc.sync.dma_start(X[0:1, 0:1, lo:hi], xr[0:1, 1:2, dlo : dlo + ncol])
                nc.sync.dma_start(
                    X[p - 1 : p, k + 1 : k + 2, lo:hi],
                    xr[p - 1 : p, k - 2 : k - 1, dlo : dlo + ncol],
                )
                # reflect cols: image col -1 := col 1 ; col w := col w-2
                if ci == 0:
                    nc.vector.tensor_copy(X[:, :, 0:1], X[:, :, 2:3])
                if ci == nchunks - 1:
                    nc.vector.tensor_copy(
                        X[:, :, wc + 1 : wc + 2], X[:, :, wc - 1 : wc]
                    )

                # prescale in place: X now holds 0.25*x
                nc.vector.tensor_scalar_mul(X[:], X[:], 0.25)

                Hq = mpool.tile([p, k + 2, wc], dt, tag="hq")
                nc.vector.tensor_add(Hq[:], X[:, :, 0:wc], X[:, :, 2 : wc + 2])
                Vq = mpool.tile([p, k, wc], dt, tag="vq")
                nc.vector.tensor_add(
                    Vq[:], X[:, 0:k, 1 : wc + 1], X[:, 2 : k + 2, 1 : wc + 1]
                )

                R = opool.tile([p, k, wc], dt, tag="r")
                G = opool.tile([p, k, wc], dt, tag="g")
                B = opool.tile([p, k, wc], dt, tag="b")

                ce, co = slice(0, wc, 2), slice(1, wc, 2)          # out/Hq/Vq cols
                cxe, cxo = slice(1, wc + 1, 2), slice(2, wc + 2, 2)  # X cols

                act = nc.scalar.activation
                tt = nc.vector.tensor_add
                # R
                act(R[:, je, ce], X[:, jxe, cxe], Copy, scale=4.0)
                act(R[:, je, co], Hq[:, jxe, co], Copy, scale=2.0)
                act(R[:, jo, ce], Vq[:, jo, ce], Copy, scale=2.0)
                tt(R[:, jo, co], Hq[:, jm_o, co], Hq[:, jp_o, co])
                # G
                tt(G[:, je, ce], Vq[:, je, ce], Hq[:, jxe, ce])
                act(G[:, je, co], X[:, jxe, cxo], Copy, scale=4.0)
                act(G[:, jo, ce], X[:, jxo, cxe], Copy, scale=4.0)
                tt(G[:, jo, co], Vq[:, jo, co], Hq[:, jxo, co])
                # B
                tt(B[:, je, ce], Hq[:, jm_e, ce], Hq[:, jp_e, ce])
                act(B[:, je, co], Vq[:, je, co], Copy, scale=2.0)
                act(B[:, jo, ce], Hq[:, jxo, ce], Copy, scale=2.0)
                act(B[:, jo, co], X[:, jxo, cxo], Copy, scale=4.0)

                for ch, plane in enumerate((R, G, B)):
                    nc.sync.dma_start(outr[ch, :, :, c0 : c0 + wc], plane[:])
    if bacc_mod is not None:
        nc.compile()
    return nc


def _get_nc():
    key = (H, W, K, WC)
    if key not in _compiled:
        import concourse.bass as bass
        import concourse.tile as tile
        from concourse import bacc, mybir

        _compiled[key] = _build(bass, tile, mybir, H, W, K, WC, bacc_mod=bacc)
    return _compiled[key]


def kernel(x: np.ndarray, kernels: np.ndarray | None = None) -> np.ndarray:
    """x: (8, 1, 2048, 2048) f32 -> (8, 3, 2048, 2048) f32."""
    from concourse.bass_utils import run_bass_kernel_spmd

    x = np.ascontiguousarray(np.asarray(x, dtype=np.float32))
    b = x.shape[0]
    assert x.shape == (b, 1, H, W) and b == N_CORES
    nc = _get_nc()
    in_maps = [{"x": x[i, 0]} for i in range(b)]
    res = run_bass_kernel_spmd(nc, in_maps, list(range(N_CORES)))
    return np.stack([res.results[i]["out"] for i in range(b)], axis=0)
